# revision 15
# baseline (speedup 1.0000x reference)
"""Multi-head causal attention (B=4,S=2048,E=1024,H=16,D=64) on 8 Trainium2 cores.

Sharding: core c handles batch b=c//2 and head-half g=c%2 (8 heads each).
Each core computes QKV projections for its heads, causal attention, and a
partial output projection with its slice of wo rows. Host sums the two
partials per batch and adds bo.

Per-core structure (all matmuls fp32r = full-rate; fp32 storage):
  phase X:  transpose x_b into x^T (PE transpose via identity), once.
  then per head-pair (pair-outer so pair p+1's projection matmuls on PE
  overlap pair p's attention exponentials on ScalarE):
    - stream w chunks, Q^T/K^T/V^T = w_chunk.T @ x^T (+bias on DVE)
    - V_aug[t, 65] via PE transpose (col 64 = ones -> softmax denominator)
    - per s-window: scores^T = K^T_tile.T @ Q^T_win (2-head row-packed via
      tile_position), exp on ScalarE (scale=1/8), causal-mask multiply on
      diagonal tiles (DVE), PV accumulation o_aug^T += V_aug_tile.T @ P^T;
      normalize via reciprocal of row 64 + K=1 broadcast matmul + DVE mul.
  phase Y: y = o^T_chunks.T @ wo_chunks accumulated over the 4 pairs.
"""

from contextlib import ExitStack

import numpy as np

B, S, E, H, D = 4, 2048, 1024, 16, 64
HH = H // 2          # heads per core
NPAIR = HH // 2      # head pairs per core
NT = S // 128        # t tiles
SW = 512             # s window width
NSW = S // SW
NEC = E // 128       # e chunks

_cache = {}


def _build(repeat=1, phases="full"):
    import concourse.bacc as bacc
    import concourse.mybir as mybir
    import concourse.tile as tile

    f32 = mybir.dt.float32
    f32r = mybir.dt.float32r
    FT = mybir.ActivationFunctionType

    nc = bacc.Bacc("TRN2", target_bir_lowering=False, debug=False,
                   enable_asserts=True, num_devices=8)

    xb = nc.dram_tensor("xb", [S, E], f32, kind="ExternalInput")
    w_dr = {p: nc.dram_tensor(f"w{p}_s", [E, HH * D], f32, kind="ExternalInput")
            for p in "qkv"}
    b_dr = {p: nc.dram_tensor(f"b{p}_s", [NPAIR, 128, 1], f32, kind="ExternalInput")
            for p in "qkv"}
    wo_dr = nc.dram_tensor("wo_s", [HH * D, E], f32, kind="ExternalInput")
    yp = nc.dram_tensor("yp", [S, E], f32, kind="ExternalOutput")

    ident_d = nc.inline_tensor(np.eye(128, dtype=np.float32), name="ident")
    mask_np = np.zeros((128, 4 * SW), np.float32)
    for k in range(4):
        mask_np[:, k * SW:(k + 1) * SW] = (
            np.arange(128)[:, None] + 128 * k <= np.arange(SW)[None, :])
    mask_d = nc.inline_tensor(mask_np, name="masks")
    ones_bc_d = nc.inline_tensor(np.ones((1, 64), np.float32), name="ones_bc")
    ones_col_d = nc.inline_tensor(np.ones((128, NT, 1), np.float32), name="ones_col")

    with tile.TileContext(nc) as tc, ExitStack() as stk:
        const = stk.enter_context(tc.tile_pool(name="const", bufs=1))
        pxt = stk.enter_context(tc.tile_pool(name="pxt", bufs=1))
        pot = stk.enter_context(tc.tile_pool(name="pot", bufs=1))
        pqkv = stk.enter_context(tc.tile_pool(name="pqkv", bufs=2))
        pva = stk.enter_context(tc.tile_pool(name="pva", bufs=1))
        pw = stk.enter_context(tc.tile_pool(name="pw", bufs=1))
        ppt = stk.enter_context(tc.tile_pool(name="ppt", bufs=2))
        pz = stk.enter_context(tc.tile_pool(name="pz", bufs=1))
        psum = stk.enter_context(tc.tile_pool(name="psum", bufs=1, space="PSUM"))

        ident = const.tile([128, 128], f32, tag="ident")
        nc.sync.dma_start(ident[:], ident_d[:])
        masks = const.tile([128, 4 * SW], f32r, tag="masks")
        nc.sync.dma_start(masks[:], mask_d[:].bitcast(f32r))
        ones_bc = const.tile([1, 64], f32, tag="ones_bc")
        nc.sync.dma_start(ones_bc[:], ones_bc_d[:])
        bias_sb = {}
        for p in "qkv":
            for pair in range(NPAIR):
                t = const.tile([128, 1], f32, tag=f"b{p}{pair}")
                nc.sync.dma_start(t[:], b_dr[p][pair])
                bias_sb[(p, pair)] = t

        for _rep in range(repeat):
            # phase X: x^T, chunk-major [128, NEC*S]
            xt = pxt.tile([128, NEC * S], f32r, tag="xt", name=f"xt{_rep}")
            with tc.tile_pool(name="pxr", bufs=4) as pxr:
                for sw in range(NSW):
                    xrs = []
                    for st4 in range(4):
                        st = sw * 4 + st4
                        xr = pxr.tile([128, E], f32, tag="xr", name=f"xr{_rep}_{st}")
                        nc.sync.dma_start(xr[:], xb[st * 128:(st + 1) * 128, :])
                        xrs.append(xr)
                    for ec in range(NEC):
                        tp = psum.tile([128, 512], f32, tag="big", bufs=2,
                                       name=f"xtr{_rep}_{sw}_{ec}")
                        for st4 in range(4):
                            nc.tensor.transpose(
                                tp[:, st4 * 128:(st4 + 1) * 128],
                                xrs[st4][:, ec * 128:(ec + 1) * 128], ident[:])
                        nc.vector.tensor_copy(
                            xt[:, ec * S + sw * 512: ec * S + sw * 512 + 512], tp[:])

            OT = [pot.tile([128, S], f32r, tag=f"ot{p}", name=f"ot{_rep}_{p}")
                  for p in range(NPAIR)]

            for pair in (range(NPAIR) if phases != "x" else ()):
                # --- QKV projections for this pair ---
                wch = pw.tile([128, 3 * NEC * 128], f32r, tag="wch",
                              name=f"wch{_rep}_{pair}")
                for pi, p in enumerate("qkv"):
                    for ec in range(NEC):
                        nc.sync.dma_start(
                            wch[:, (pi * NEC + ec) * 128:(pi * NEC + ec + 1) * 128],
                            w_dr[p][ec * 128:(ec + 1) * 128,
                                    pair * 128:(pair + 1) * 128].bitcast(f32r))
                QKV = {}
                for pi, p in enumerate("qkv"):
                    dst = pqkv.tile([128, S], f32r if p != "v" else f32,
                                    tag=f"{p}t", bufs=(1 if p == "v" else 2),
                                    name=f"{p}t{_rep}_{pair}")
                    QKV[p] = dst
                    for sw in range(NSW):
                        acc = psum.tile([128, 512], f32, tag="big", bufs=2,
                                        name=f"acc{_rep}_{pair}_{p}_{sw}")
                        for ec in range(NEC):
                            nc.tensor.matmul(
                                acc[:],
                                wch[:, (pi * NEC + ec) * 128:(pi * NEC + ec + 1) * 128],
                                xt[:, ec * S + sw * SW: ec * S + sw * SW + SW],
                                start=(ec == 0), stop=(ec == NEC - 1))
                        nc.vector.tensor_scalar_add(
                            dst[:, sw * SW:(sw + 1) * SW], acc[:],
                            bias_sb[(p, pair)][:])
                QT, KT, VT = QKV["q"], QKV["k"], QKV["v"]

                # --- V_aug for the two heads of this pair ---
                VAt = pva.tile([128, 2 * NT * 65], f32r, tag="va",
                               name=f"va{_rep}_{pair}")
                VA = [VAt[:, hp * NT * 65:(hp + 1) * NT * 65] for hp in range(2)]
                for hp in range(2):
                    nc.sync.dma_start(
                        VA[hp].rearrange("p (t c) -> p t c", c=65)[:, :, 64:65],
                        ones_col_d[:].bitcast(f32r))
                for tt2 in range(NT // 2):
                    tp = psum.tile([128, 512], f32, tag="big", bufs=2,
                                   name=f"vtr{_rep}_{pair}_{tt2}")
                    for q in range(2):
                        nc.tensor.transpose(
                            tp[:, q * 256: q * 256 + 128],
                            VT[:, (2 * tt2 + q) * 128:(2 * tt2 + q + 1) * 128],
                            ident[:])
                    # tp[:, q*256+hp*64 : +64] = V of head hp, t-tile 2*tt2+q
                    for hp in range(2):
                        nc.vector.tensor_copy(
                            VA[hp].rearrange("p (t c) -> p t c", c=65)[
                                :, 2 * tt2: 2 * tt2 + 2, 0:64],
                            tp[:].rearrange("p (t x) -> p t x", t=2)[
                                :, :, hp * 64: hp * 64 + 64])

                # --- attention for this pair ---
                for sw in (range(NSW) if phases in ("xqa", "full") else ()):
                    o0 = psum.tile([65, 512], f32, tag="o0", bufs=1,
                                   name=f"o0_{_rep}_{pair}_{sw}")
                    o1 = psum.tile([65, 512], f32, tag="o1", bufs=1,
                                   name=f"o1_{_rep}_{pair}_{sw}")
                    ntt = 4 * sw + 4

                    def scores(tt, pair=pair, sw=sw, QT=QT, KT=KT):
                        sc = psum.tile([128, 1024], f32, tag="sc", bufs=2,
                                       name=f"sc_{_rep}_{pair}_{sw}_{tt}")
                        nc.tensor.matmul(
                            sc[:, 0:512], KT[0:64, tt * 128:(tt + 1) * 128],
                            QT[0:64, sw * SW:(sw + 1) * SW],
                            start=True, stop=True, tile_position=(0, 0))
                        nc.tensor.matmul(
                            sc[:, 512:1024], KT[64:128, tt * 128:(tt + 1) * 128],
                            QT[64:128, sw * SW:(sw + 1) * SW],
                            start=True, stop=True, tile_position=(64, 0))
                        return sc

                    def exp_pv(tt, sc, pair=pair, sw=sw, o0=o0, o1=o1,
                               ntt=ntt, VA=VA):
                        pt = ppt.tile([128, 1024], f32r, tag="pt",
                                      name=f"pt_{_rep}_{pair}_{sw}_{tt}")
                        nc.scalar.activation(pt[:], sc[:], FT.Exp, scale=0.125)
                        k = tt - 4 * sw
                        for hp, o_ps in ((0, o0), (1, o1)):
                            if k >= 0:
                                nc.vector.tensor_mul(
                                    pt[:, hp * 512:(hp + 1) * 512],
                                    pt[:, hp * 512:(hp + 1) * 512],
                                    masks[:, k * SW:(k + 1) * SW])
                            nc.tensor.matmul(
                                o_ps[:], VA[hp][:, tt * 65: tt * 65 + 65],
                                pt[:, hp * 512:(hp + 1) * 512],
                                start=(tt == 0), stop=(tt == ntt - 1))

                    prev = None
                    for tt in range(ntt):
                        cur = scores(tt)
                        if prev is not None:
                            exp_pv(tt - 1, prev)
                        prev = cur
                    exp_pv(ntt - 1, prev)

                    for hp, o_ps in ((0, o0), (1, o1)):
                        zr = pz.tile([1, 512], f32, tag=f"zr{hp}",
                                     name=f"zr{_rep}_{pair}_{sw}_{hp}")
                        nc.vector.reciprocal(zr[:], o_ps[64:65, :])
                        zb = psum.tile([64, 512], f32, tag="big", bufs=2,
                                       name=f"zb{_rep}_{pair}_{sw}_{hp}")
                        nc.tensor.matmul(zb[:], ones_bc[:], zr[:], start=True, stop=True)
                        zbs = pz.tile([64, 512], f32, tag=f"zbs{hp}",
                                      name=f"zbs{_rep}_{pair}_{sw}_{hp}")
                        nc.vector.tensor_copy(zbs[:], zb[:])
                        nc.vector.tensor_mul(
                            OT[pair][hp * 64:(hp + 1) * 64, sw * SW:(sw + 1) * SW],
                            o_ps[0:64, :], zbs[:])

            # phase Y: output projection (WO/ysb reuse the dead QKV slots)
            if phases != "full":
                continue
            WO = [pqkv.tile([128, E], f32r, tag=("qt" if p < 2 else "kt"),
                            name=f"wo{_rep}_{p}") for p in range(NPAIR)]
            for p in range(NPAIR):
                nc.sync.dma_start(
                    WO[p][:], wo_dr[p * 128:(p + 1) * 128, :].bitcast(f32r))
            for st in range(NT):
                for ew in range(E // 512):
                    y_ps = psum.tile([128, 512], f32, tag="big", bufs=2,
                                     name=f"y{_rep}_{st}_{ew}")
                    for p in range(NPAIR):
                        nc.tensor.matmul(
                            y_ps[:], OT[p][:, st * 128:(st + 1) * 128],
                            WO[p][:, ew * 512:(ew + 1) * 512],
                            start=(p == 0), stop=(p == NPAIR - 1))
                    ysb = pz.tile([128, 512], f32, tag="ysb", bufs=2,
                                  name=f"ysb{_rep}_{st}_{ew}")
                    nc.vector.tensor_copy(ysb[:], y_ps[:])
                    nc.sync.dma_start(
                        yp[st * 128:(st + 1) * 128, ew * 512:(ew + 1) * 512],
                        ysb[:])

    nc.compile()
    return nc


def _in_maps(x, wq, bq, wk, bk, wv, bv, wo):
    maps = []
    for c in range(8):
        b, g = divmod(c, 2)
        hs = slice(g * HH, (g + 1) * HH)

        def wslice(w):
            return np.ascontiguousarray(
                np.asarray(w[hs]).transpose(1, 0, 2).reshape(E, HH * D),
                dtype=np.float32)

        def bslice(bias):
            return np.ascontiguousarray(
                np.asarray(bias[hs]).reshape(NPAIR, 128, 1), dtype=np.float32)

        maps.append({
            "xb": np.ascontiguousarray(np.asarray(x[b]), dtype=np.float32),
            "wq_s": wslice(wq), "wk_s": wslice(wk), "wv_s": wslice(wv),
            "bq_s": bslice(bq), "bk_s": bslice(bk), "bv_s": bslice(bv),
            "wo_s": np.ascontiguousarray(
                np.asarray(wo[g * HH * D:(g + 1) * HH * D, :]), dtype=np.float32),
        })
    return maps


def kernel(x, wq, bq, wk, bk, wv, bv, wo, bo, _trace=False):
    from concourse.bass_utils import run_bass_kernel_spmd

    if "nc" not in _cache:
        _cache["nc"] = _build()
    nc = _cache["nc"]

    maps = _in_maps(x, wq, bq, wk, bk, wv, bv, wo)
    res = run_bass_kernel_spmd(nc, maps, core_ids=list(range(8)), trace=_trace)
    _cache["last_result"] = res

    y = np.zeros((B, S, E), np.float32)
    for c in range(8):
        y[c // 2] += res.results[c]["yp"]
    y += np.asarray(bo, dtype=np.float32)
    return y


# revision 19
# speedup vs baseline: 1.1744x; 1.1744x over previous
"""Multi-head causal attention (B=4,S=2048,E=1024,H=16,D=64) on 8 Trainium2 cores.

Sharding: core c handles batch b=c//2 and head-half g=c%2 (8 heads each).
Each core computes QKV projections for its heads, causal attention, and a
partial output projection with its slice of wo rows. Host sums the two
partials per batch and adds bo.

Per-core structure (all matmuls fp32r = full-rate; fp32 storage):
  phase X:  transpose x_b into x^T (PE transpose via identity), once.
  then per head-pair (pair-outer so pair p+1's projection matmuls on PE
  overlap pair p's attention exponentials on ScalarE):
    - stream w chunks, Q^T/K^T/V^T = w_chunk.T @ x^T (+bias on DVE)
    - V_aug[t, 65] via PE transpose (col 64 = ones -> softmax denominator)
    - per s-window: scores^T = K^T_tile.T @ Q^T_win (2-head row-packed via
      tile_position), exp on ScalarE (scale=1/8), causal-mask multiply on
      diagonal tiles (DVE), PV accumulation o_aug^T += V_aug_tile.T @ P^T;
      normalize via reciprocal of row 64 + K=1 broadcast matmul + DVE mul.
  phase Y: y = o^T_chunks.T @ wo_chunks accumulated over the 4 pairs.
"""

from contextlib import ExitStack

import numpy as np

B, S, E, H, D = 4, 2048, 1024, 16, 64
HH = H // 2          # heads per core
NPAIR = HH // 2      # head pairs per core
NT = S // 128        # t tiles
SW = 512             # s window width
NSW = S // SW
NEC = E // 128       # e chunks

_cache = {}


def _build(repeat=1, phases="full", pva_bufs=1, ppt_bufs=3):
    import concourse.bacc as bacc
    import concourse.mybir as mybir
    import concourse.tile as tile

    f32 = mybir.dt.float32
    f32r = mybir.dt.float32r
    FT = mybir.ActivationFunctionType

    nc = bacc.Bacc("TRN2", target_bir_lowering=False, debug=False,
                   enable_asserts=True, num_devices=8)

    xb = nc.dram_tensor("xb", [S, E], f32, kind="ExternalInput")
    w_dr = {p: nc.dram_tensor(f"w{p}_s", [E, HH * D], f32, kind="ExternalInput")
            for p in "qkv"}
    b_dr = {p: nc.dram_tensor(f"b{p}_s", [NPAIR, 128, 1], f32, kind="ExternalInput")
            for p in "qkv"}
    wo_dr = nc.dram_tensor("wo_s", [HH * D, E], f32, kind="ExternalInput")
    yp = nc.dram_tensor("yp", [S, E], f32, kind="ExternalOutput")

    ident_d = nc.inline_tensor(np.eye(128, dtype=np.float32), name="ident")
    mask_np = np.zeros((128, 4 * SW), np.float32)
    for k in range(4):
        mask_np[:, k * SW:(k + 1) * SW] = (
            np.arange(128)[:, None] + 128 * k <= np.arange(SW)[None, :])
    mask_d = nc.inline_tensor(mask_np, name="masks")
    ones_bc_d = nc.inline_tensor(np.ones((1, 64), np.float32), name="ones_bc")
    ones_col_d = nc.inline_tensor(np.ones((128, NT, 1), np.float32), name="ones_col")

    with tile.TileContext(nc) as tc, ExitStack() as stk:
        const = stk.enter_context(tc.tile_pool(name="const", bufs=1))
        pxt = stk.enter_context(tc.tile_pool(name="pxt", bufs=1))
        pot = stk.enter_context(tc.tile_pool(name="pot", bufs=1))
        pqkv = stk.enter_context(tc.tile_pool(name="pqkv", bufs=2))
        pva = stk.enter_context(tc.tile_pool(name="pva", bufs=pva_bufs))
        pw = stk.enter_context(tc.tile_pool(name="pw", bufs=1))
        ppt = stk.enter_context(tc.tile_pool(name="ppt", bufs=ppt_bufs))
        pz = stk.enter_context(tc.tile_pool(name="pz", bufs=1))
        psum = stk.enter_context(tc.tile_pool(name="psum", bufs=1, space="PSUM"))

        ident = const.tile([128, 128], f32, tag="ident")
        nc.sync.dma_start(ident[:], ident_d[:])
        masks = const.tile([128, 4 * SW], f32r, tag="masks")
        nc.sync.dma_start(masks[:], mask_d[:].bitcast(f32r))
        ones_bc = const.tile([1, 64], f32, tag="ones_bc")
        nc.sync.dma_start(ones_bc[:], ones_bc_d[:])
        bias_sb = {}
        for p in "qkv":
            for pair in range(NPAIR):
                t = const.tile([128, 1], f32, tag=f"b{p}{pair}")
                nc.sync.dma_start(t[:], b_dr[p][pair])
                bias_sb[(p, pair)] = t

        for _rep in range(repeat):
            # phase X: x^T, chunk-major [128, NEC*S]
            xt = pxt.tile([128, NEC * S], f32r, tag="xt", name=f"xt{_rep}")
            with tc.tile_pool(name="pxr", bufs=4) as pxr:
                for sw in range(NSW):
                    xrs = []
                    for st4 in range(4):
                        st = sw * 4 + st4
                        xr = pxr.tile([128, E], f32, tag="xr", name=f"xr{_rep}_{st}")
                        nc.sync.dma_start(xr[:], xb[st * 128:(st + 1) * 128, :])
                        xrs.append(xr)
                    for ec in range(NEC):
                        tp = psum.tile([128, 512], f32, tag="big", bufs=2,
                                       name=f"xtr{_rep}_{sw}_{ec}")
                        for st4 in range(4):
                            nc.tensor.transpose(
                                tp[:, st4 * 128:(st4 + 1) * 128],
                                xrs[st4][:, ec * 128:(ec + 1) * 128], ident[:])
                        nc.vector.tensor_copy(
                            xt[:, ec * S + sw * 512: ec * S + sw * 512 + 512], tp[:])

            OT = [pot.tile([128, S], f32r, tag=f"ot{p}", name=f"ot{_rep}_{p}")
                  for p in range(NPAIR)]

            for pair in (range(NPAIR) if phases != "x" else ()):
                # --- QKV projections for this pair ---
                wch = pw.tile([128, 3 * NEC * 128], f32r, tag="wch",
                              name=f"wch{_rep}_{pair}")
                for pi, p in enumerate("qkv"):
                    for ec in range(NEC):
                        nc.sync.dma_start(
                            wch[:, (pi * NEC + ec) * 128:(pi * NEC + ec + 1) * 128],
                            w_dr[p][ec * 128:(ec + 1) * 128,
                                    pair * 128:(pair + 1) * 128].bitcast(f32r))
                QKV = {}
                for pi, p in enumerate("qkv"):
                    dst = pqkv.tile([128, S], f32r if p != "v" else f32,
                                    tag=f"{p}t", bufs=(1 if p == "v" else 2),
                                    name=f"{p}t{_rep}_{pair}")
                    QKV[p] = dst
                    for sw in range(NSW):
                        acc = psum.tile([128, 512], f32, tag="big", bufs=2,
                                        name=f"acc{_rep}_{pair}_{p}_{sw}")
                        for ec in range(NEC):
                            nc.tensor.matmul(
                                acc[:],
                                wch[:, (pi * NEC + ec) * 128:(pi * NEC + ec + 1) * 128],
                                xt[:, ec * S + sw * SW: ec * S + sw * SW + SW],
                                start=(ec == 0), stop=(ec == NEC - 1))
                        nc.vector.tensor_scalar_add(
                            dst[:, sw * SW:(sw + 1) * SW], acc[:],
                            bias_sb[(p, pair)][:])
                QT, KT, VT = QKV["q"], QKV["k"], QKV["v"]

                # --- V_aug for the two heads of this pair ---
                VAt = pva.tile([128, 2 * NT * 65], f32r, tag="va",
                               name=f"va{_rep}_{pair}")
                VA = [VAt[:, hp * NT * 65:(hp + 1) * NT * 65] for hp in range(2)]
                for hp in range(2):
                    nc.sync.dma_start(
                        VA[hp].rearrange("p (t c) -> p t c", c=65)[:, :, 64:65],
                        ones_col_d[:].bitcast(f32r))
                for tt2 in range(NT // 2):
                    tp = psum.tile([128, 512], f32, tag="big", bufs=2,
                                   name=f"vtr{_rep}_{pair}_{tt2}")
                    for q in range(2):
                        nc.tensor.transpose(
                            tp[:, q * 256: q * 256 + 128],
                            VT[:, (2 * tt2 + q) * 128:(2 * tt2 + q + 1) * 128],
                            ident[:])
                    # tp[:, q*256+hp*64 : +64] = V of head hp, t-tile 2*tt2+q
                    for hp in range(2):
                        nc.vector.tensor_copy(
                            VA[hp].rearrange("p (t c) -> p t c", c=65)[
                                :, 2 * tt2: 2 * tt2 + 2, 0:64],
                            tp[:].rearrange("p (t x) -> p t x", t=2)[
                                :, :, hp * 64: hp * 64 + 64])

                # --- attention for this pair ---
                for sw in (range(NSW) if phases in ("xqa", "full") else ()):
                    o0 = psum.tile([65, 512], f32, tag="o0", bufs=1,
                                   name=f"o0_{_rep}_{pair}_{sw}")
                    o1 = psum.tile([65, 512], f32, tag="o1", bufs=1,
                                   name=f"o1_{_rep}_{pair}_{sw}")
                    ntt = 4 * sw + 4

                    def scores(tt, pair=pair, sw=sw, QT=QT, KT=KT):
                        sc = psum.tile([128, 1024], f32, tag="sc", bufs=2,
                                       name=f"sc_{_rep}_{pair}_{sw}_{tt}")
                        nc.tensor.matmul(
                            sc[:, 0:512], KT[0:64, tt * 128:(tt + 1) * 128],
                            QT[0:64, sw * SW:(sw + 1) * SW],
                            start=True, stop=True, tile_position=(0, 0))
                        nc.tensor.matmul(
                            sc[:, 512:1024], KT[64:128, tt * 128:(tt + 1) * 128],
                            QT[64:128, sw * SW:(sw + 1) * SW],
                            start=True, stop=True, tile_position=(64, 0))
                        return sc

                    def exp_pv(tt, sc, pair=pair, sw=sw, o0=o0, o1=o1,
                               ntt=ntt, VA=VA):
                        pt = ppt.tile([128, 1024], f32r, tag="pt",
                                      name=f"pt_{_rep}_{pair}_{sw}_{tt}")
                        nc.scalar.activation(pt[:], sc[:], FT.Exp, scale=0.125)
                        k = tt - 4 * sw
                        for hp, o_ps in ((0, o0), (1, o1)):
                            if k >= 0:
                                nc.vector.tensor_mul(
                                    pt[:, hp * 512:(hp + 1) * 512],
                                    pt[:, hp * 512:(hp + 1) * 512],
                                    masks[:, k * SW:(k + 1) * SW])
                            nc.tensor.matmul(
                                o_ps[:], VA[hp][:, tt * 65: tt * 65 + 65],
                                pt[:, hp * 512:(hp + 1) * 512],
                                start=(tt == 0), stop=(tt == ntt - 1))

                    prev = None
                    for tt in range(ntt):
                        cur = scores(tt)
                        if prev is not None:
                            exp_pv(tt - 1, prev)
                        prev = cur
                    exp_pv(ntt - 1, prev)

                    for hp, o_ps in ((0, o0), (1, o1)):
                        zr = pz.tile([1, 512], f32, tag=f"zr{hp}",
                                     name=f"zr{_rep}_{pair}_{sw}_{hp}")
                        nc.vector.reciprocal(zr[:], o_ps[64:65, :])
                        zb = psum.tile([64, 512], f32, tag="big", bufs=2,
                                       name=f"zb{_rep}_{pair}_{sw}_{hp}")
                        nc.tensor.matmul(zb[:], ones_bc[:], zr[:], start=True, stop=True)
                        zbs = pz.tile([64, 512], f32, tag=f"zbs{hp}",
                                      name=f"zbs{_rep}_{pair}_{sw}_{hp}")
                        nc.vector.tensor_copy(zbs[:], zb[:])
                        nc.vector.tensor_mul(
                            OT[pair][hp * 64:(hp + 1) * 64, sw * SW:(sw + 1) * SW],
                            o_ps[0:64, :], zbs[:])

            # phase Y: output projection (WO/ysb reuse the dead QKV slots)
            if phases != "full":
                continue
            WO = [pqkv.tile([128, E], f32r, tag=("qt" if p < 2 else "kt"),
                            name=f"wo{_rep}_{p}") for p in range(NPAIR)]
            for p in range(NPAIR):
                nc.sync.dma_start(
                    WO[p][:], wo_dr[p * 128:(p + 1) * 128, :].bitcast(f32r))
            for st in range(NT):
                for ew in range(E // 512):
                    y_ps = psum.tile([128, 512], f32, tag="big", bufs=2,
                                     name=f"y{_rep}_{st}_{ew}")
                    for p in range(NPAIR):
                        nc.tensor.matmul(
                            y_ps[:], OT[p][:, st * 128:(st + 1) * 128],
                            WO[p][:, ew * 512:(ew + 1) * 512],
                            start=(p == 0), stop=(p == NPAIR - 1))
                    ysb = pz.tile([128, 512], f32, tag="ysbp", bufs=1,
                                  name=f"ysb{_rep}_{st}_{ew}")
                    nc.vector.tensor_copy(ysb[:], y_ps[:])
                    nc.sync.dma_start(
                        yp[st * 128:(st + 1) * 128, ew * 512:(ew + 1) * 512],
                        ysb[:])

    nc.compile()
    return nc


def _in_maps(x, wq, bq, wk, bk, wv, bv, wo):
    maps = []
    for c in range(8):
        b, g = divmod(c, 2)
        hs = slice(g * HH, (g + 1) * HH)

        def wslice(w):
            return np.ascontiguousarray(
                np.asarray(w[hs]).transpose(1, 0, 2).reshape(E, HH * D),
                dtype=np.float32)

        def bslice(bias):
            return np.ascontiguousarray(
                np.asarray(bias[hs]).reshape(NPAIR, 128, 1), dtype=np.float32)

        maps.append({
            "xb": np.ascontiguousarray(np.asarray(x[b]), dtype=np.float32),
            "wq_s": wslice(wq), "wk_s": wslice(wk), "wv_s": wslice(wv),
            "bq_s": bslice(bq), "bk_s": bslice(bk), "bv_s": bslice(bv),
            "wo_s": np.ascontiguousarray(
                np.asarray(wo[g * HH * D:(g + 1) * HH * D, :]), dtype=np.float32),
        })
    return maps


def kernel(x, wq, bq, wk, bk, wv, bv, wo, bo, _trace=False):
    from concourse.bass_utils import run_bass_kernel_spmd

    if "nc" not in _cache:
        _cache["nc"] = _build()
    nc = _cache["nc"]

    maps = _in_maps(x, wq, bq, wk, bk, wv, bv, wo)
    res = run_bass_kernel_spmd(nc, maps, core_ids=list(range(8)), trace=_trace)
    _cache["last_result"] = res

    y = np.zeros((B, S, E), np.float32)
    for c in range(8):
        y[c // 2] += res.results[c]["yp"]
    y += np.asarray(bo, dtype=np.float32)
    return y


# revision 21
# speedup vs baseline: 1.6439x; 1.3997x over previous
"""Multi-head causal attention (B=4,S=2048,E=1024,H=16,D=64) on 8 Trainium2 cores.

Sharding: core c handles batch b=c//2 and head-half g=c%2 (8 heads each).
Each core computes QKV projections for its heads, causal attention, and a
partial output projection with its slice of wo rows. Host sums the two
partials per batch and adds bo.

Per-core structure (all matmuls fp32r = full-rate; fp32 storage):
  phase X:  transpose x_b into x^T (PE transpose via identity), once.
  then per head-pair (pair-outer so pair p+1's projection matmuls on PE
  overlap pair p's attention exponentials on ScalarE):
    - stream w chunks, Q^T/K^T/V^T = w_chunk.T @ x^T (+bias on DVE)
    - V_aug[t, 65] via PE transpose (col 64 = ones -> softmax denominator)
    - per s-window: scores^T = K^T_tile.T @ Q^T_win (2-head row-packed via
      tile_position), exp on ScalarE (scale=1/8), causal-mask multiply on
      diagonal tiles (DVE), PV accumulation o_aug^T += V_aug_tile.T @ P^T;
      normalize via reciprocal of row 64 + K=1 broadcast matmul + DVE mul.
  phase Y: y = o^T_chunks.T @ wo_chunks accumulated over the 4 pairs.
"""

from contextlib import ExitStack

import numpy as np

B, S, E, H, D = 4, 2048, 1024, 16, 64
HH = H // 2          # heads per core
NPAIR = HH // 2      # head pairs per core
NT = S // 128        # t tiles
SW = 512             # s window width
NSW = S // SW
NEC = E // 128       # e chunks

_cache = {}


def _build(repeat=1, phases="full", pva_bufs=1, ppt_bufs=2):
    import concourse.bacc as bacc
    import concourse.mybir as mybir
    import concourse.tile as tile

    f32 = mybir.dt.float32
    f32r = mybir.dt.float32r
    FT = mybir.ActivationFunctionType

    nc = bacc.Bacc("TRN2", target_bir_lowering=False, debug=False,
                   enable_asserts=True, num_devices=8)

    xb = nc.dram_tensor("xb", [S, E], f32, kind="ExternalInput")
    w_dr = {p: nc.dram_tensor(f"w{p}_s", [E, HH * D], f32, kind="ExternalInput")
            for p in "qkv"}
    b_dr = {p: nc.dram_tensor(f"b{p}_s", [NPAIR, 128, 1], f32, kind="ExternalInput")
            for p in "qkv"}
    wo_dr = nc.dram_tensor("wo_s", [HH * D, E], f32, kind="ExternalInput")
    yp = nc.dram_tensor("yp", [S, E], f32, kind="ExternalOutput")

    ident_d = nc.inline_tensor(np.eye(128, dtype=np.float32), name="ident")
    mask_np = np.zeros((128, 4 * SW), np.float32)
    for k in range(4):
        mask_np[:, k * SW:(k + 1) * SW] = (
            np.arange(128)[:, None] + 128 * k <= np.arange(SW)[None, :])
    mask_d = nc.inline_tensor(mask_np, name="masks")
    ones_bc_d = nc.inline_tensor(np.ones((1, 64), np.float32), name="ones_bc")
    ones_col_d = nc.inline_tensor(np.ones((128, NT, 1), np.float32), name="ones_col")

    with tile.TileContext(nc) as tc, ExitStack() as stk:
        const = stk.enter_context(tc.tile_pool(name="const", bufs=1))
        pxt = stk.enter_context(tc.tile_pool(name="pxt", bufs=1))
        pot = stk.enter_context(tc.tile_pool(name="pot", bufs=1))
        pqkv = stk.enter_context(tc.tile_pool(name="pqkv", bufs=2))
        pva = stk.enter_context(tc.tile_pool(name="pva", bufs=pva_bufs))
        pw = stk.enter_context(tc.tile_pool(name="pw", bufs=1))
        ppt = stk.enter_context(tc.tile_pool(name="ppt", bufs=ppt_bufs))
        pz = stk.enter_context(tc.tile_pool(name="pz", bufs=1))
        psum = stk.enter_context(tc.tile_pool(name="psum", bufs=1, space="PSUM"))

        ident = const.tile([128, 128], f32, tag="ident")
        nc.sync.dma_start(ident[:], ident_d[:])
        masks = const.tile([128, 4 * SW], f32r, tag="masks")
        nc.sync.dma_start(masks[:], mask_d[:].bitcast(f32r))
        ones_bc = const.tile([1, 64], f32, tag="ones_bc")
        nc.sync.dma_start(ones_bc[:], ones_bc_d[:])
        bias_sb = {}
        for p in "qkv":
            for pair in range(NPAIR):
                t = const.tile([128, 1], f32, tag=f"b{p}{pair}")
                nc.sync.dma_start(t[:], b_dr[p][pair])
                bias_sb[(p, pair)] = t

        for _rep in range(repeat):
            # phase X: x^T, chunk-major [128, NEC*S]
            xt = pxt.tile([128, NEC * S], f32r, tag="xt", name=f"xt{_rep}")
            with tc.tile_pool(name="pxr", bufs=4) as pxr:
                for sw in range(NSW):
                    xrs = []
                    for st4 in range(4):
                        st = sw * 4 + st4
                        xr = pxr.tile([128, E], f32, tag="xr", name=f"xr{_rep}_{st}")
                        nc.sync.dma_start(xr[:], xb[st * 128:(st + 1) * 128, :])
                        xrs.append(xr)
                    for ec in range(NEC):
                        tp = psum.tile([128, 512], f32, tag="big", bufs=2,
                                       name=f"xtr{_rep}_{sw}_{ec}")
                        for st4 in range(4):
                            nc.tensor.transpose(
                                tp[:, st4 * 128:(st4 + 1) * 128],
                                xrs[st4][:, ec * 128:(ec + 1) * 128], ident[:])
                        nc.vector.tensor_copy(
                            xt[:, ec * S + sw * 512: ec * S + sw * 512 + 512], tp[:])

            OT = [pot.tile([128, S], f32r, tag=f"ot{p}", name=f"ot{_rep}_{p}")
                  for p in range(NPAIR)]

            deferred_norm = []
            for pair in (range(NPAIR) if phases != "x" else ()):
                # --- QKV projections for this pair ---
                wch = pw.tile([128, 3 * NEC * 128], f32r, tag="wch",
                              name=f"wch{_rep}_{pair}")
                for pi, p in enumerate("qkv"):
                    for ec in range(NEC):
                        nc.sync.dma_start(
                            wch[:, (pi * NEC + ec) * 128:(pi * NEC + ec + 1) * 128],
                            w_dr[p][ec * 128:(ec + 1) * 128,
                                    pair * 128:(pair + 1) * 128].bitcast(f32r))
                QKV = {}
                for pi, p in enumerate("qkv"):
                    dst = pqkv.tile([128, S], f32r if p != "v" else f32,
                                    tag=f"{p}t", bufs=(1 if p == "v" else 2),
                                    name=f"{p}t{_rep}_{pair}")
                    QKV[p] = dst
                    for sw in range(NSW):
                        acc = psum.tile([128, 512], f32, tag="big", bufs=2,
                                        name=f"acc{_rep}_{pair}_{p}_{sw}")
                        for ec in range(NEC):
                            nc.tensor.matmul(
                                acc[:],
                                wch[:, (pi * NEC + ec) * 128:(pi * NEC + ec + 1) * 128],
                                xt[:, ec * S + sw * SW: ec * S + sw * SW + SW],
                                start=(ec == 0), stop=(ec == NEC - 1))
                        nc.vector.tensor_scalar_add(
                            dst[:, sw * SW:(sw + 1) * SW], acc[:],
                            bias_sb[(p, pair)][:])
                QT, KT, VT = QKV["q"], QKV["k"], QKV["v"]

                # --- V_aug for the two heads of this pair ---
                VAt = pva.tile([128, 2 * NT * 65], f32r, tag="va",
                               name=f"va{_rep}_{pair}")
                VA = [VAt[:, hp * NT * 65:(hp + 1) * NT * 65] for hp in range(2)]
                for hp in range(2):
                    nc.sync.dma_start(
                        VA[hp].rearrange("p (t c) -> p t c", c=65)[:, :, 64:65],
                        ones_col_d[:].bitcast(f32r))
                for tt2 in range(NT // 2):
                    tp = psum.tile([128, 512], f32, tag="big", bufs=2,
                                   name=f"vtr{_rep}_{pair}_{tt2}")
                    for q in range(2):
                        nc.tensor.transpose(
                            tp[:, q * 256: q * 256 + 128],
                            VT[:, (2 * tt2 + q) * 128:(2 * tt2 + q + 1) * 128],
                            ident[:])
                    # tp[:, q*256+hp*64 : +64] = V of head hp, t-tile 2*tt2+q
                    for hp in range(2):
                        nc.vector.tensor_copy(
                            VA[hp].rearrange("p (t c) -> p t c", c=65)[
                                :, 2 * tt2: 2 * tt2 + 2, 0:64],
                            tp[:].rearrange("p (t x) -> p t x", t=2)[
                                :, :, hp * 64: hp * 64 + 64])

                # --- attention for this pair ---
                for sw in (range(NSW) if phases in ("xqa", "full") else ()):
                    o0 = psum.tile([65, 512], f32, tag="o0", bufs=1,
                                   name=f"o0_{_rep}_{pair}_{sw}")
                    o1 = psum.tile([65, 512], f32, tag="o1", bufs=1,
                                   name=f"o1_{_rep}_{pair}_{sw}")
                    ntt = 4 * sw + 4

                    def scores(tt, pair=pair, sw=sw, QT=QT, KT=KT):
                        sc = psum.tile([128, 1024], f32, tag="sc", bufs=2,
                                       name=f"sc_{_rep}_{pair}_{sw}_{tt}")
                        nc.tensor.matmul(
                            sc[:, 0:512], KT[0:64, tt * 128:(tt + 1) * 128],
                            QT[0:64, sw * SW:(sw + 1) * SW],
                            start=True, stop=True, tile_position=(0, 0))
                        nc.tensor.matmul(
                            sc[:, 512:1024], KT[64:128, tt * 128:(tt + 1) * 128],
                            QT[64:128, sw * SW:(sw + 1) * SW],
                            start=True, stop=True, tile_position=(64, 0))
                        return sc

                    def exp_pv(tt, sc, pair=pair, sw=sw, o0=o0, o1=o1,
                               ntt=ntt, VA=VA):
                        pt = ppt.tile([128, 1024], f32r, tag="pt",
                                      name=f"pt_{_rep}_{pair}_{sw}_{tt}")
                        nc.scalar.activation(pt[:], sc[:], FT.Exp, scale=0.125)
                        k = tt - 4 * sw
                        for hp, o_ps in ((0, o0), (1, o1)):
                            if k >= 0:
                                nc.vector.tensor_mul(
                                    pt[:, hp * 512:(hp + 1) * 512],
                                    pt[:, hp * 512:(hp + 1) * 512],
                                    masks[:, k * SW:(k + 1) * SW])
                            nc.tensor.matmul(
                                o_ps[:], VA[hp][:, tt * 65: tt * 65 + 65],
                                pt[:, hp * 512:(hp + 1) * 512],
                                start=(tt == 0), stop=(tt == ntt - 1))

                    prev = None
                    for tt in range(ntt):
                        cur = scores(tt)
                        if prev is not None:
                            exp_pv(tt - 1, prev)
                        prev = cur
                    exp_pv(ntt - 1, prev)

                    # free the o banks fast: one copy out, normalize later
                    for hp, o_ps in ((0, o0), (1, o1)):
                        osv = pz.tile([65, 512], f32, tag=f"osv{hp}", bufs=2,
                                      name=f"osv{_rep}_{pair}_{sw}_{hp}")
                        nc.vector.tensor_copy(osv[:], o_ps[:])
                        deferred_norm.append((pair, sw, hp, osv))
                    while len(deferred_norm) > 2:
                        npair, nsw, nhp, osv = deferred_norm.pop(0)
                        zr = pz.tile([1, 512], f32, tag=f"zr{nhp}",
                                     name=f"zr{_rep}_{npair}_{nsw}_{nhp}")
                        nc.vector.reciprocal(zr[:], osv[64:65, :])
                        zb = psum.tile([64, 512], f32, tag="big", bufs=2,
                                       name=f"zb{_rep}_{npair}_{nsw}_{nhp}")
                        nc.tensor.matmul(zb[:], ones_bc[:], zr[:], start=True, stop=True)
                        zbs = pz.tile([64, 512], f32, tag=f"zbs{nhp}",
                                      name=f"zbs{_rep}_{npair}_{nsw}_{nhp}")
                        nc.vector.tensor_copy(zbs[:], zb[:])
                        nc.vector.tensor_mul(
                            OT[npair][nhp * 64:(nhp + 1) * 64, nsw * SW:(nsw + 1) * SW],
                            osv[0:64, :], zbs[:])

            for npair, nsw, nhp, osv in deferred_norm:
                zr = pz.tile([1, 512], f32, tag=f"zr{nhp}",
                             name=f"zrf{_rep}_{npair}_{nsw}_{nhp}")
                nc.vector.reciprocal(zr[:], osv[64:65, :])
                zb = psum.tile([64, 512], f32, tag="big", bufs=2,
                               name=f"zbf{_rep}_{npair}_{nsw}_{nhp}")
                nc.tensor.matmul(zb[:], ones_bc[:], zr[:], start=True, stop=True)
                zbs = pz.tile([64, 512], f32, tag=f"zbs{nhp}",
                              name=f"zbsf{_rep}_{npair}_{nsw}_{nhp}")
                nc.vector.tensor_copy(zbs[:], zb[:])
                nc.vector.tensor_mul(
                    OT[npair][nhp * 64:(nhp + 1) * 64, nsw * SW:(nsw + 1) * SW],
                    osv[0:64, :], zbs[:])
            deferred_norm = []

            # phase Y: output projection (WO/ysb reuse the dead QKV slots)
            if phases != "full":
                continue
            WO = [pqkv.tile([128, E], f32r, tag=("qt" if p < 2 else "kt"),
                            name=f"wo{_rep}_{p}") for p in range(NPAIR)]
            for p in range(NPAIR):
                nc.sync.dma_start(
                    WO[p][:], wo_dr[p * 128:(p + 1) * 128, :].bitcast(f32r))
            for st in range(NT):
                for ew in range(E // 512):
                    y_ps = psum.tile([128, 512], f32, tag="big", bufs=2,
                                     name=f"y{_rep}_{st}_{ew}")
                    for p in range(NPAIR):
                        nc.tensor.matmul(
                            y_ps[:], OT[p][:, st * 128:(st + 1) * 128],
                            WO[p][:, ew * 512:(ew + 1) * 512],
                            start=(p == 0), stop=(p == NPAIR - 1))
                    ysb = pz.tile([128, 512], f32, tag="ysbp", bufs=1,
                                  name=f"ysb{_rep}_{st}_{ew}")
                    nc.vector.tensor_copy(ysb[:], y_ps[:])
                    nc.sync.dma_start(
                        yp[st * 128:(st + 1) * 128, ew * 512:(ew + 1) * 512],
                        ysb[:])

    nc.compile()
    return nc


def _in_maps(x, wq, bq, wk, bk, wv, bv, wo):
    maps = []
    for c in range(8):
        b, g = divmod(c, 2)
        hs = slice(g * HH, (g + 1) * HH)

        def wslice(w):
            return np.ascontiguousarray(
                np.asarray(w[hs]).transpose(1, 0, 2).reshape(E, HH * D),
                dtype=np.float32)

        def bslice(bias):
            return np.ascontiguousarray(
                np.asarray(bias[hs]).reshape(NPAIR, 128, 1), dtype=np.float32)

        maps.append({
            "xb": np.ascontiguousarray(np.asarray(x[b]), dtype=np.float32),
            "wq_s": wslice(wq), "wk_s": wslice(wk), "wv_s": wslice(wv),
            "bq_s": bslice(bq), "bk_s": bslice(bk), "bv_s": bslice(bv),
            "wo_s": np.ascontiguousarray(
                np.asarray(wo[g * HH * D:(g + 1) * HH * D, :]), dtype=np.float32),
        })
    return maps


def kernel(x, wq, bq, wk, bk, wv, bv, wo, bo, _trace=False):
    from concourse.bass_utils import run_bass_kernel_spmd

    if "nc" not in _cache:
        _cache["nc"] = _build()
    nc = _cache["nc"]

    maps = _in_maps(x, wq, bq, wk, bk, wv, bv, wo)
    res = run_bass_kernel_spmd(nc, maps, core_ids=list(range(8)), trace=_trace)
    _cache["last_result"] = res

    y = np.zeros((B, S, E), np.float32)
    for c in range(8):
        y[c // 2] += res.results[c]["yp"]
    y += np.asarray(bo, dtype=np.float32)
    return y
